# revision 1
# baseline (speedup 1.0000x reference)
"""FISTA compressed-sensing kernel for Trainium2 (8 NeuronCores, SPMD).

Problem: for each of 64 patches (x3 channels), run 200 FISTA iterations of
    min_x 0.5||A x - b||^2 + lam||x||_1,   A: (81, 5184)
Sharding: pure data-parallel over the batch — 8 patches x 3 channels = 24
columns per core; A replicated.

Per-core formulation (column matrix Y: (5184, 24)):
    Ay   = A @ Y                      (81, 24)
    G    = A^T @ Ay - Atb             (5184, 24)   [Atb folded into the
                                                    matmul via 24 extra
                                                    contraction rows]
    Z    = Y - mu*G
    Xn   = soft_threshold(Z, lam*mu)
    Y'   = Xn + coef_i * (Xn - X)

Layout: D=5184 padded to 5248 = 41*128; state tiles [128, kt, 24] with
d = kt*128 + p. Column groups g0 = ktiles 0..20, g1 = 21..40 so each
gradient group fits one PSUM bank.
"""

import os

import numpy as np

import concourse.bass as bass
import concourse.mybir as mybir
import concourse.tile as tile
from concourse.bass_utils import run_bass_kernel_spmd

F32 = mybir.dt.float32

M = 81            # measurements (9x9 camera patch)
D = 5184          # atoms (72x72 upsampled grid)
KT = 41           # 128-row tiles covering D (padded to 5248)
DP = KT * 128     # 5248
NCORES = 8
B = 64
BPC = B // NCORES           # 8 patches per core
N = BPC * 3                 # 24 state columns per core
ITERS = int(os.environ.get("FISTA_ITERS", "200"))
KA = 128                    # augmented contraction dim for matmul2
CT0 = 96                    # partition row where the -Atb^T block starts (32-aligned)
G0, G1 = 21, 20             # ktiles per column group (504 / 480 psum cols)

_CACHE = {}


def _legalize_waits(nc):
    """This walrus build accepts at most ONE semaphore wait per instruction
    (setupSyncWait: 'Too many sync wait commands'). Tile emits multi-wait
    instructions; split the excess waits onto injected same-engine NoOps
    placed immediately before the instruction (engine queues are FIFO, so
    semantics are identical)."""
    n = 0
    for fn in nc.m.functions:
        for bb in fn.blocks:
            insts = bb.instructions
            out = []
            changed = False
            for ins in insts:
                si = ins.sync_info
                ow = list(si.on_wait) if si is not None else []
                if len(ow) > 1 and ins.engine is not None:
                    for w in ow[:-1]:
                        n += 1
                        out.append(mybir.InstNoOp(
                            name=f"I-waitnop-{n}",
                            engine=ins.engine,
                            ins=[],
                            outs=[],
                            debug=ins.debug,
                            sync_info=mybir.SyncInfo(on_wait=[w], on_update=[]),
                        ))
                    ins.sync_info = mybir.SyncInfo(
                        on_wait=[ow[-1]], on_update=list(si.on_update))
                    changed = True
                out.append(ins)
            if changed:
                bb.instructions = out
    return n


def _fista_coefs(iters):
    t = 1.0
    coefs = []
    for _ in range(iters):
        t_new = (1.0 + float(np.sqrt(1.0 + 4.0 * t * t))) / 2.0
        coefs.append((t - 1.0) / t_new)
        t = t_new
    return coefs


def _build(mu_s, thr, iters):
    """Build the Bass module (same program for all 8 cores)."""
    nc = bass.Bass()
    coefs = _fista_coefs(iters)

    # DRAM parameters (per core): A^T tiles, A tiles, measurement matrix b.
    at_d = nc.declare_dram_parameter("at", [128, KT, M], F32, isOutput=False)
    a_d = nc.declare_dram_parameter("a", [128, KT, 128], F32, isOutput=False)
    b_d = nc.declare_dram_parameter("b", [M, N], F32, isOutput=False)
    ay0_d = nc.declare_dram_parameter("ayinit", [128, N], F32, isOutput=False)
    xout_d = nc.declare_dram_parameter("xout", [128, KT, N], F32, isOutput=True)

    with tile.TileContext(nc) as tc:
        with (
            tc.tile_pool(name="weights", bufs=1) as wpool,
            tc.tile_pool(name="state", bufs=1) as spool,
            tc.tile_pool(name="tmp", bufs=2) as tpool,
            tc.tile_pool(name="psum_ay", bufs=2, space="PSUM") as ppool_ay,
            tc.tile_pool(name="psum_gp", bufs=2, space="PSUM") as ppool_gp,
            tc.tile_pool(name="psum_ct", bufs=2, space="PSUM") as ppool_ct,
        ):
            # --- persistent SBUF tensors -------------------------------
            at_sb = wpool.tile([128, KT, M], F32)      # lhsT for matmul1
            w2_sb = wpool.tile([KA, KT, 128], F32)     # lhsT for matmul2
            ay_sb = wpool.tile([KA, N], F32)           # rhs for matmul2
            y_sb = [spool.tile([128, G0, N], F32, tag="y0", name="y0"),
                    spool.tile([128, G1, N], F32, tag="y1", name="y1")]
            # x ping-pong: x_sb[s][g]
            x_sb = [[spool.tile([128, G0, N], F32, tag=f"x{s}0", name=f"x{s}0"),
                     spool.tile([128, G1, N], F32, tag=f"x{s}1", name=f"x{s}1")]
                    for s in range(2)]

            nc.sync.dma_start(out=at_sb[:], in_=at_d[:])
            nc.sync.dma_start(out=w2_sb[:], in_=a_d[:])

            b_sb = wpool.tile([M, N], F32)
            nc.sync.dma_start(out=b_sb[:], in_=b_d[:])

            negthr = wpool.tile([128, 1], F32)
            nc.vector.memset(negthr[:], -thr)

            # rhs init: zeros with identity block at rows CT0..CT0+N
            nc.sync.dma_start(out=ay_sb[:], in_=ay0_d[:])

            # initial state: x = y = 0
            for g in range(2):
                nc.vector.memset(y_sb[g][:], 0.0)
                nc.vector.memset(x_sb[1][g][:], 0.0)

            # --- fold -Atb^T into rows CT0..CT0+N of w2 ----------------
            # ct = b^T A  (24, 5248), computed in chunks of 4 ktiles (512),
            # staged in SBUF then written into w2 with ONE instruction so
            # downstream LDWEIGHTS carry few sync waits.
            ct_stage = wpool.tile([N, KT * 128], F32)
            for c4 in range((KT + 3) // 4):
                k0 = c4 * 4
                nk = min(4, KT - k0)
                ct_ps = ppool_ct.tile([N, 4 * 128], F32, tag="ct")
                nc.tensor.matmul(
                    ct_ps[:, : nk * 128],
                    b_sb[:],
                    w2_sb[0:M, k0 : k0 + nk, :].rearrange("k a b -> k (a b)"),
                    start=True,
                    stop=True,
                )
                nc.vector.tensor_scalar_mul(
                    ct_stage[:, k0 * 128 : (k0 + nk) * 128],
                    ct_ps[:, : nk * 128],
                    -1.0,
                )
            nc.vector.tensor_copy(
                w2_sb[CT0 : CT0 + N, :, :].rearrange("k a b -> k (a b)"),
                ct_stage[:],
            )

            # group g -> (ktile offset, ktile count)
            gidx = [(0, G0), (G0, G1)]

            # --- FISTA iterations --------------------------------------
            for i in range(iters):
                cur, prev = i % 2, (i + 1) % 2

                # matmul1: Ay = A @ Y  -> psum (81, 24)
                ay_ps = ppool_ay.tile([M, N], F32, tag="ay")
                for kt in range(KT):
                    g, j = (0, kt) if kt < G0 else (1, kt - G0)
                    nc.tensor.matmul(
                        ay_ps[:],
                        at_sb[:, kt, :],
                        y_sb[g][:, j, :],
                        start=(kt == 0),
                        stop=(kt == KT - 1),
                    )
                nc.vector.tensor_copy(ay_sb[0:M, :], ay_ps[:])

                # matmul2 + elementwise, per column group
                for g in range(2):
                    k0, ng = gidx[g]
                    gp = ppool_gp.tile([128, ng, N], F32, tag=f"gp{g}")
                    for j in range(ng):
                        nc.tensor.matmul(
                            gp[:, j, :],
                            w2_sb[:, k0 + j, :],
                            ay_sb[:],
                            start=True,
                            stop=True,
                        )
                    # z = y - mu*(A^T Ay - Atb) = (gp * -mu) + y
                    z = tpool.tile([128, ng, N], F32, tag=f"z{g}")
                    nc.vector.scalar_tensor_tensor(
                        out=z[:],
                        in0=gp[:],
                        scalar=-mu_s,
                        in1=y_sb[g][:],
                        op0=mybir.AluOpType.mult,
                        op1=mybir.AluOpType.add,
                    )
                    # soft threshold: xn = relu(z - thr) - relu(-z - thr)
                    p = tpool.tile([128, ng, N], F32, tag=f"p{g}")
                    q = tpool.tile([128, ng, N], F32, tag=f"q{g}")
                    nc.scalar.activation(
                        out=p[:], in_=z[:],
                        func=mybir.ActivationFunctionType.Relu,
                        bias=negthr[:], scale=1.0,
                    )
                    nc.scalar.activation(
                        out=q[:], in_=z[:],
                        func=mybir.ActivationFunctionType.Relu,
                        bias=negthr[:], scale=-1.0,
                    )
                    xn = x_sb[cur][g]
                    nc.vector.tensor_sub(xn[:], p[:], q[:])
                    if i == 0:
                        # coef_0 = 0 -> y = xn
                        nc.gpsimd.tensor_copy(y_sb[g][:], xn[:])
                    else:
                        d = tpool.tile([128, ng, N], F32, tag=f"d{g}")
                        nc.gpsimd.tensor_sub(d[:], xn[:], x_sb[prev][g][:])
                        nc.vector.scalar_tensor_tensor(
                            out=y_sb[g][:],
                            in0=d[:],
                            scalar=coefs[i],
                            in1=xn[:],
                            op0=mybir.AluOpType.mult,
                            op1=mybir.AluOpType.add,
                        )

            # --- write back final x ------------------------------------
            fin = (iters - 1) % 2
            nc.sync.dma_start(out=xout_d[:, 0:G0, :], in_=x_sb[fin][0][:])
            nc.sync.dma_start(out=xout_d[:, G0:KT, :], in_=x_sb[fin][1][:])

    _legalize_waits(nc)
    return nc


def _prep_inputs(inp, A):
    """Host-side shard/reshape: returns per-core input maps."""
    A = np.asarray(A, np.float32)
    A_pad = np.zeros((M, DP), np.float32)
    A_pad[:, :D] = A
    a_tiles = np.zeros((128, KT, 128), np.float32)
    a_tiles[:M] = A_pad.reshape(M, KT, 128)
    ay_init = np.zeros((128, N), np.float32)
    ay_init[CT0 : CT0 + N] = np.eye(N, dtype=np.float32)
    at_tiles = np.ascontiguousarray(
        A_pad.T.reshape(KT, 128, M).transpose(1, 0, 2))  # [128, KT, M]

    inp = np.asarray(inp, np.float32)
    in_maps = []
    for c in range(NCORES):
        chunk = inp[c * BPC : (c + 1) * BPC]            # (8, 81, 3)
        b_mat = np.ascontiguousarray(chunk.transpose(1, 0, 2).reshape(M, N))
        in_maps.append({"at": at_tiles, "a": a_tiles, "b": b_mat,
                        "ayinit": ay_init})
    return in_maps


def _unshard(results):
    outs = []
    for c in range(NCORES):
        xo = np.asarray(results[c]["xout"])              # [128, KT, N]
        x_dn = xo.transpose(1, 0, 2).reshape(DP, N)[:D]  # (5184, 24)
        outs.append(x_dn.reshape(72, 72, BPC, 3).transpose(2, 0, 1, 3))
    return np.concatenate(outs, 0).astype(np.float32)    # (64, 72, 72, 3)


def _run(inp, A, lam, mu, trace=False):
    mu_s = float(np.asarray(mu).reshape(-1)[0])
    thr = float(np.asarray(lam).reshape(-1)[0]) * mu_s
    key = (mu_s, thr, ITERS)
    if key not in _CACHE:
        _CACHE[key] = _build(mu_s, thr, ITERS)
    nc = _CACHE[key]
    in_maps = _prep_inputs(inp, A)
    res = run_bass_kernel_spmd(nc, in_maps, list(range(NCORES)), trace=trace)
    return _unshard(res.results), res


def kernel(inp, A, lam, mu):
    out, _ = _run(inp, A, lam, mu)
    return out



# revision 9
# speedup vs baseline: 79.2658x; 79.2658x over previous
"""FISTA compressed-sensing kernel for Trainium2 (8 NeuronCores, SPMD).

Problem: for each of 64 patches (x3 channels), run 200 FISTA iterations of
    min_x 0.5||A x - b||^2 + lam||x||_1,   A: (81, 5184)
Sharding: pure data-parallel over the batch — 8 patches x 3 channels = 24
columns per core; A replicated.

Per-core formulation (column matrix Y: (5184, 24)):
    Ay   = A @ Y                      (81, 24)
    G    = A^T @ Ay - Atb             (5184, 24)   [Atb folded into the
                                                    matmul via 24 extra
                                                    contraction rows]
    Z    = Y - mu*G
    Xn   = soft_threshold(Z, lam*mu)
    Y'   = Xn + coef_i * (Xn - X)

Layout: D=5184 padded to 5248 = 41*128; state tiles [128, kt, 24] with
d = kt*128 + p. Column groups g0 = ktiles 0..20, g1 = 21..40 so each
gradient group fits one PSUM bank.
"""

import os

import numpy as np

import concourse.bass as bass
import concourse.mybir as mybir
import concourse.tile as tile
from concourse.bass_utils import run_bass_kernel_spmd

F32 = mybir.dt.float32

M = 81            # measurements (9x9 camera patch)
D = 5184          # atoms (72x72 upsampled grid)
KT = 41           # 128-row tiles covering D (padded to 5248)
DP = KT * 128     # 5248
NCORES = 8
B = 64
BPC = B // NCORES           # 8 patches per core
N = BPC * 3                 # 24 state columns per core
ITERS = int(os.environ.get("FISTA_ITERS", "200"))
KA = 128                    # augmented contraction dim for matmul2
CT0 = 96                    # partition row where the -Atb^T block starts (32-aligned)
G0, G1 = 21, 20             # ktiles per column group (504 / 480 psum cols)
COEF_PAD = 256              # fixed momentum-coef table size (>= any ITERS used)

_CACHE = {}


def _legalize_waits(nc):
    """This walrus build accepts at most ONE semaphore wait per instruction
    (setupSyncWait: 'Too many sync wait commands'). Tile emits multi-wait
    instructions; split the excess waits onto injected same-engine NoOps
    placed immediately before the instruction (engine queues are FIFO, so
    semantics are identical)."""
    n = 0
    for fn in nc.m.functions:
        for bb in fn.blocks:
            insts = bb.instructions
            out = []
            changed = False
            for ins in insts:
                si = ins.sync_info
                ow = list(si.on_wait) if si is not None else []
                if len(ow) > 1 and ins.engine is not None:
                    for w in ow[:-1]:
                        n += 1
                        out.append(mybir.InstNoOp(
                            name=f"I-waitnop-{n}",
                            engine=ins.engine,
                            ins=[],
                            outs=[],
                            debug=ins.debug,
                            sync_info=mybir.SyncInfo(on_wait=[w], on_update=[]),
                        ))
                    ins.sync_info = mybir.SyncInfo(
                        on_wait=[ow[-1]], on_update=list(si.on_update))
                    changed = True
                out.append(ins)
            if changed:
                bb.instructions = out
    return n


def _fista_coefs(iters):
    t = 1.0
    coefs = []
    for _ in range(iters):
        t_new = (1.0 + float(np.sqrt(1.0 + 4.0 * t * t))) / 2.0
        coefs.append((t - 1.0) / t_new)
        t = t_new
    return coefs


def _build(mu_s, thr, iters, coef_pad=COEF_PAD):
    """Build the Bass module (same program for all 8 cores)."""
    assert iters % 2 == 0 and iters <= coef_pad
    nc = bass.Bass()

    # DRAM parameters (per core): A^T tiles, A tiles, measurement matrix b.
    at_d = nc.declare_dram_parameter("at", [128, KT, M], F32, isOutput=False)
    a_d = nc.declare_dram_parameter("a", [128, KT, 128], F32, isOutput=False)
    b_d = nc.declare_dram_parameter("b", [M, N], F32, isOutput=False)
    ay0_d = nc.declare_dram_parameter("ayinit", [128, N], F32, isOutput=False)
    coef_d = nc.declare_dram_parameter("coef", [128, coef_pad], F32,
                                       isOutput=False)
    xout_d = nc.declare_dram_parameter("xout", [128, KT, N], F32, isOutput=True)

    with tile.TileContext(nc) as tc:
        with (
            tc.tile_pool(name="weights", bufs=1) as wpool,
            tc.tile_pool(name="state", bufs=1) as spool,
            tc.tile_pool(name="tmp", bufs=2) as tpool,
            tc.tile_pool(name="psum_ay", bufs=2, space="PSUM") as ppool_ay,
            tc.tile_pool(name="psum_gp", bufs=2, space="PSUM") as ppool_gp,
            tc.tile_pool(name="psum_ct", bufs=2, space="PSUM") as ppool_ct,
        ):
            # --- persistent SBUF tensors -------------------------------
            at_sb = wpool.tile([128, KT, M], F32)      # lhsT for matmul1
            w2_sb = wpool.tile([KA, KT, 128], F32)     # lhsT for matmul2
            ay_sb = wpool.tile([KA, N], F32)           # rhs for matmul2
            y_sb = [spool.tile([128, G0, N], F32, tag="y0", name="y0"),
                    spool.tile([128, G1, N], F32, tag="y1", name="y1")]
            # x ping-pong: x_sb[s][g]
            x_sb = [[spool.tile([128, G0, N], F32, tag=f"x{s}0", name=f"x{s}0"),
                     spool.tile([128, G1, N], F32, tag=f"x{s}1", name=f"x{s}1")]
                    for s in range(2)]

            nc.sync.dma_start(out=at_sb[:], in_=at_d[:])
            nc.sync.dma_start(out=w2_sb[:], in_=a_d[:])

            b_sb = wpool.tile([M, N], F32)
            nc.sync.dma_start(out=b_sb[:], in_=b_d[:])

            negthr = wpool.tile([128, 1], F32)
            nc.vector.memset(negthr[:], -thr)

            # rhs init: zeros with identity block at rows CT0..CT0+N
            nc.sync.dma_start(out=ay_sb[:], in_=ay0_d[:])

            # initial state: x = y = 0
            for g in range(2):
                nc.vector.memset(y_sb[g][:], 0.0)
                nc.vector.memset(x_sb[1][g][:], 0.0)

            # --- fold -Atb^T into rows CT0..CT0+N of w2 ----------------
            # ct = b^T A  (24, 5248), computed in chunks of 4 ktiles (512),
            # staged in SBUF then written into w2 with ONE instruction so
            # downstream LDWEIGHTS carry few sync waits.
            ct_stage = wpool.tile([N, KT * 128], F32)
            for c4 in range((KT + 3) // 4):
                k0 = c4 * 4
                nk = min(4, KT - k0)
                ct_ps = ppool_ct.tile([N, 4 * 128], F32, tag="ct")
                nc.tensor.matmul(
                    ct_ps[:, : nk * 128],
                    b_sb[:],
                    w2_sb[0:M, k0 : k0 + nk, :].rearrange("k a b -> k (a b)"),
                    start=True,
                    stop=True,
                )
                nc.vector.tensor_scalar_mul(
                    ct_stage[:, k0 * 128 : (k0 + nk) * 128],
                    ct_ps[:, : nk * 128],
                    -1.0,
                )
            nc.vector.tensor_copy(
                w2_sb[CT0 : CT0 + N, :, :].rearrange("k a b -> k (a b)"),
                ct_stage[:],
            )

            # coef table [128, coef_pad] (rows identical), for dynamic
            # lookup inside the hardware loop
            coef_sb = wpool.tile([128, coef_pad], F32)
            nc.sync.dma_start(out=coef_sb[:], in_=coef_d[:])

            # group g -> (ktile offset, ktile count)
            gidx = [(0, G0), (G0, G1)]

            # --- FISTA iterations: hardware loop, 2 per trip -----------
            assert iters % 2 == 0
            with tc.For_i(0, iters, 2) as it:
                for phase in range(2):
                    cur, prev = phase, 1 - phase
                    c_ap = coef_sb[:, bass.ds(it + phase, 1)]

                    # matmul1: Ay = A @ Y  -> psum (81, 24)
                    ay_ps = ppool_ay.tile([M, N], F32, tag="ay")
                    for kt in range(KT):
                        g, j = (0, kt) if kt < G0 else (1, kt - G0)
                        nc.tensor.matmul(
                            ay_ps[:],
                            at_sb[:, kt, :],
                            y_sb[g][:, j, :],
                            start=(kt == 0),
                            stop=(kt == KT - 1),
                        )
                    nc.vector.tensor_copy(ay_sb[0:M, :], ay_ps[:])

                    # matmul2 + elementwise, per column group
                    for g in range(2):
                        k0, ng = gidx[g]
                        gp = ppool_gp.tile([128, ng, N], F32, tag=f"gp{g}")
                        for j in range(ng):
                            nc.tensor.matmul(
                                gp[:, j, :],
                                w2_sb[:, k0 + j, :],
                                ay_sb[:],
                                start=True,
                                stop=True,
                            )
                        # z = y - mu*(A^T Ay - Atb) = (gp * -mu) + y
                        z = tpool.tile([128, ng, N], F32, tag=f"z{g}")
                        nc.vector.scalar_tensor_tensor(
                            out=z[:],
                            in0=gp[:],
                            scalar=-mu_s,
                            in1=y_sb[g][:],
                            op0=mybir.AluOpType.mult,
                            op1=mybir.AluOpType.add,
                        )
                        # soft threshold: xn = relu(z - thr) - relu(-z - thr)
                        p = tpool.tile([128, ng, N], F32, tag=f"p{g}")
                        q = tpool.tile([128, ng, N], F32, tag=f"q{g}")
                        nc.scalar.activation(
                            out=p[:], in_=z[:],
                            func=mybir.ActivationFunctionType.Relu,
                            bias=negthr[:], scale=1.0,
                        )
                        nc.scalar.activation(
                            out=q[:], in_=z[:],
                            func=mybir.ActivationFunctionType.Relu,
                            bias=negthr[:], scale=-1.0,
                        )
                        xn = x_sb[cur][g]
                        nc.vector.tensor_sub(xn[:], p[:], q[:])
                        # y = xn + coef[i]*(xn - x_prev); coef[0] = 0 makes
                        # the first iteration reduce to y = xn (x_prev = 0)
                        d = tpool.tile([128, ng, N], F32, tag=f"d{g}")
                        nc.gpsimd.tensor_sub(d[:], xn[:], x_sb[prev][g][:])
                        nc.vector.scalar_tensor_tensor(
                            out=y_sb[g][:],
                            in0=d[:],
                            scalar=c_ap,
                            in1=xn[:],
                            op0=mybir.AluOpType.mult,
                            op1=mybir.AluOpType.add,
                        )

            # --- write back final x ------------------------------------
            fin = (iters - 1) % 2
            nc.sync.dma_start(out=xout_d[:, 0:G0, :], in_=x_sb[fin][0][:])
            nc.sync.dma_start(out=xout_d[:, G0:KT, :], in_=x_sb[fin][1][:])

    _legalize_waits(nc)
    return nc


def _prep_inputs(inp, A, coef_pad=COEF_PAD):
    """Host-side shard/reshape: returns per-core input maps."""
    A = np.asarray(A, np.float32)
    A_pad = np.zeros((M, DP), np.float32)
    A_pad[:, :D] = A
    a_tiles = np.zeros((128, KT, 128), np.float32)
    a_tiles[:M] = A_pad.reshape(M, KT, 128)
    ay_init = np.zeros((128, N), np.float32)
    ay_init[CT0 : CT0 + N] = np.eye(N, dtype=np.float32)
    at_tiles = np.ascontiguousarray(
        A_pad.T.reshape(KT, 128, M).transpose(1, 0, 2))  # [128, KT, M]

    coef_tab = np.tile(
        np.asarray(_fista_coefs(coef_pad), np.float32)[None, :], (128, 1))

    inp = np.asarray(inp, np.float32)
    in_maps = []
    for c in range(NCORES):
        chunk = inp[c * BPC : (c + 1) * BPC]            # (8, 81, 3)
        b_mat = np.ascontiguousarray(chunk.transpose(1, 0, 2).reshape(M, N))
        in_maps.append({"at": at_tiles, "a": a_tiles, "b": b_mat,
                        "ayinit": ay_init, "coef": coef_tab})
    return in_maps


def _unshard(results):
    outs = []
    for c in range(NCORES):
        xo = np.asarray(results[c]["xout"])              # [128, KT, N]
        x_dn = xo.transpose(1, 0, 2).reshape(DP, N)[:D]  # (5184, 24)
        outs.append(x_dn.reshape(72, 72, BPC, 3).transpose(2, 0, 1, 3))
    return np.concatenate(outs, 0).astype(np.float32)    # (64, 72, 72, 3)


def _run(inp, A, lam, mu, trace=False):
    mu_s = float(np.asarray(mu).reshape(-1)[0])
    thr = float(np.asarray(lam).reshape(-1)[0]) * mu_s
    key = (mu_s, thr, ITERS)
    if key not in _CACHE:
        _CACHE[key] = _build(mu_s, thr, ITERS)
    nc = _CACHE[key]
    in_maps = _prep_inputs(inp, A)
    res = run_bass_kernel_spmd(nc, in_maps, list(range(NCORES)), trace=trace)
    return _unshard(res.results), res


def kernel(inp, A, lam, mu):
    out, _ = _run(inp, A, lam, mu)
    return out



# revision 34
# speedup vs baseline: 237.1729x; 2.9921x over previous
"""FISTA compressed-sensing kernel for Trainium2 (8 NeuronCores, SPMD).

Problem: for each of 64 patches (x3 channels), run 200 FISTA iterations of
    min_x 0.5||A x - b||^2 + lam||x||_1,   A: (81, 5184)
Sharding: pure data-parallel over the batch — 8 patches x 3 channels = 24
columns per core; A replicated.

Per-core formulation (column matrix Y: (5184, 24)):
    Ay   = A @ Y                      (81, 24)
    G    = A^T @ Ay - Atb             (5184, 24)   [Atb folded into the
                                                    matmul via 24 extra
                                                    contraction rows]
    Z    = Y - mu*G
    Xn   = soft_threshold(Z, lam*mu)
    Y'   = Xn + coef_i * (Xn - X)

Layout: D=5184 padded to 5248 = 41*128; state tiles [128, kt, 24] with
d = kt*128 + p. Ktiles are split into GROUPS so each gradient group fits
one PSUM bank and the per-iteration elementwise tail stays short.

The 200 iterations run as a hardware For_i loop (UNROLL iterations per
trip), with the FISTA momentum coefficient computed on-device via its
scalar recurrence — so the NEFF size is independent of the iteration
count and per-call NEFF-load overhead stays constant.
"""

import os

import numpy as np

import concourse.bass as bass
import concourse.mybir as mybir
import concourse.tile as tile
from concourse.bass_utils import run_bass_kernel_spmd

F32 = mybir.dt.float32

M = 81            # measurements (9x9 camera patch)
D = 5184          # atoms (72x72 upsampled grid)
KT = 41           # 128-row tiles covering D (padded to 5248)
DP = KT * 128     # 5248
NCORES = 8
B = 64
BPC = B // NCORES           # 8 patches per core
N = BPC * 3                 # 24 state columns per core
ITERS = int(os.environ.get("FISTA_ITERS", "200"))
KA = 128                    # augmented contraction dim for matmul2
CT0 = 96                    # partition row where the -Atb^T block starts (32-aligned)
# ktile split into column groups: each gradient group fits one PSUM
# bank (<=512 f32/partition) and groups pipeline matmul2 with elementwise
GROUPS = [(0, 21), (21, 20)]
UNROLL = 4                  # FISTA iterations per hardware-loop trip

_CACHE = {}


def _legalize_waits(nc):
    """This walrus build accepts at most ONE semaphore wait per instruction
    (setupSyncWait: 'Too many sync wait commands'). Tile emits multi-wait
    instructions; split the excess waits onto injected same-engine NoOps
    placed immediately before the instruction (engine queues are FIFO, so
    semantics are identical)."""
    n = 0
    for fn in nc.m.functions:
        for bb in fn.blocks:
            insts = bb.instructions
            out = []
            changed = False
            for ins in insts:
                si = ins.sync_info
                ow = list(si.on_wait) if si is not None else []
                if len(ow) > 1 and ins.engine is not None:
                    for w in ow[:-1]:
                        n += 1
                        out.append(mybir.InstNoOp(
                            name=f"I-waitnop-{n}",
                            engine=ins.engine,
                            ins=[],
                            outs=[],
                            debug=ins.debug,
                            sync_info=mybir.SyncInfo(on_wait=[w], on_update=[]),
                        ))
                    ins.sync_info = mybir.SyncInfo(
                        on_wait=[ow[-1]], on_update=list(si.on_update))
                    changed = True
                out.append(ins)
            if changed:
                bb.instructions = out
    return n


def _build(mu_s, thr, iters):
    """Build the Bass module (same program for all 8 cores)."""
    assert iters % 2 == 0
    nc = bass.Bass()

    # DRAM parameters (per core): A^T tiles, A tiles, measurement matrix b.
    at_d = nc.declare_dram_parameter("at", [128, KT, M], F32, isOutput=False)
    a_d = nc.declare_dram_parameter("a", [128, KT, 128], F32, isOutput=False)
    b_d = nc.declare_dram_parameter("b", [M, N], F32, isOutput=False)
    ay0_d = nc.declare_dram_parameter("ayinit", [128, N], F32, isOutput=False)
    xout_d = nc.declare_dram_parameter("xout", [128, KT, N], F32, isOutput=True)

    with tile.TileContext(nc) as tc:
        with (
            tc.tile_pool(name="weights", bufs=1) as wpool,
            tc.tile_pool(name="state", bufs=1) as spool,
            tc.tile_pool(name="tmp", bufs=2) as tpool,
            tc.tile_pool(name="psum_ay", bufs=2, space="PSUM") as ppool_ay,
            tc.tile_pool(name="psum_gp", bufs=2, space="PSUM") as ppool_gp,
            tc.tile_pool(name="psum_ct", bufs=2, space="PSUM") as ppool_ct,
        ):
            # --- persistent SBUF tensors -------------------------------
            at_sb = wpool.tile([128, KT, M], F32)      # lhsT for matmul1
            w2_sb = wpool.tile([KA, KT, 128], F32)     # lhsT for matmul2
            ay_sb = wpool.tile([KA, N], F32)           # rhs for matmul2
            y_sb = [spool.tile([128, cnt, N], F32, tag=f"y{g}", name=f"y{g}")
                    for g, (off, cnt) in enumerate(GROUPS)]
            # x ping-pong: x_sb[s][g]
            x_sb = [[spool.tile([128, cnt, N], F32, tag=f"x{s}{g}",
                                name=f"x{s}{g}")
                     for g, (off, cnt) in enumerate(GROUPS)]
                    for s in range(2)]

            nc.sync.dma_start(out=at_sb[:], in_=at_d[:])
            nc.sync.dma_start(out=w2_sb[:], in_=a_d[:])

            b_sb = wpool.tile([M, N], F32)
            nc.sync.dma_start(out=b_sb[:], in_=b_d[:])

            negthr = wpool.tile([128, 1], F32)
            nc.vector.memset(negthr[:], -thr)

            # rhs init: zeros with identity block at rows CT0..CT0+N
            nc.sync.dma_start(out=ay_sb[:], in_=ay0_d[:])

            # initial state: x = y = 0
            for g in range(len(GROUPS)):
                nc.vector.memset(y_sb[g][:], 0.0)
                nc.vector.memset(x_sb[1][g][:], 0.0)

            # --- fold -Atb^T into rows CT0..CT0+N of w2 ----------------
            # ct = b^T A  (24, 5248), computed in chunks of 4 ktiles (512),
            # staged in SBUF then written into w2 with ONE instruction so
            # downstream LDWEIGHTS carry few sync waits.
            ct_stage = wpool.tile([N, KT * 128], F32)
            for c4 in range((KT + 3) // 4):
                k0 = c4 * 4
                nk = min(4, KT - k0)
                ct_ps = ppool_ct.tile([N, 4 * 128], F32, tag="ct")
                nc.tensor.matmul(
                    ct_ps[:, : nk * 128],
                    b_sb[:],
                    w2_sb[0:M, k0 : k0 + nk, :].rearrange("k a b -> k (a b)"),
                    start=True,
                    stop=True,
                )
                nc.vector.tensor_scalar_mul(
                    ct_stage[:, k0 * 128 : (k0 + nk) * 128],
                    ct_ps[:, : nk * 128],
                    -1.0,
                )
            nc.vector.tensor_copy(
                w2_sb[CT0 : CT0 + N, :, :].rearrange("k a b -> k (a b)"),
                ct_stage[:],
            )

            # momentum coefficient state for the on-device recurrence:
            # t_{k+1} = (1 + sqrt(1 + 4 t_k^2)) / 2,  c_k = (t_k - 1)/t_{k+1}
            # t ping-pongs by iteration parity; c in its own tile per phase.
            t_sb = [wpool.tile([128, 1], F32, name="t0"),
                    wpool.tile([128, 1], F32, name="t1")]
            c_sb = [wpool.tile([128, 1], F32, name="c0"),
                    wpool.tile([128, 1], F32, name="c1")]
            tmp1 = wpool.tile([128, 1], F32, name="tmp1")
            nc.vector.memset(t_sb[0][:], 1.0)

            # ktile -> group index lookup
            kt2g = {}
            for g, (off, cnt) in enumerate(GROUPS):
                for j in range(cnt):
                    kt2g[off + j] = (g, j)

            # --- FISTA iterations: hardware loop, UNROLL per trip ------
            unroll = UNROLL if iters % UNROLL == 0 else 2
            assert iters % unroll == 0
            with tc.For_i(0, iters, unroll,
                          hint_engines=(mybir.EngineType.PE,)) as it:
                for phase in range(unroll):
                    cur, prev = phase % 2, 1 - phase % 2
                    tp, tn = t_sb[cur], t_sb[prev]
                    # coef chain (tiny [128,1] ops, off the critical path):
                    # tmp1 = 4*t^2 + 1 ; tn = (sqrt(tmp1)+1)/2 ;
                    # tmp1 = 1/tn ; c = (t-1)*tmp1
                    nc.vector.tensor_scalar(
                        tmp1[:], tp[:], tp[:], None, mybir.AluOpType.mult)
                    nc.scalar.activation(
                        out=tn[:], in_=tmp1[:],
                        func=mybir.ActivationFunctionType.Sqrt,
                        bias=1.0, scale=4.0)
                    nc.vector.tensor_scalar(
                        tn[:], tn[:], 1.0, 0.5,
                        mybir.AluOpType.add, mybir.AluOpType.mult)
                    nc.vector.reciprocal(tmp1[:], tn[:])
                    nc.vector.scalar_tensor_tensor(
                        out=c_sb[cur][:], in0=tp[:], scalar=1.0,
                        in1=tmp1[:],
                        op0=mybir.AluOpType.subtract,
                        op1=mybir.AluOpType.mult)
                    c_ap = c_sb[cur][:]

                    # matmul1: Ay = A @ Y  -> psum (81, 24)
                    ay_ps = ppool_ay.tile([M, N], F32, tag="ay")
                    for kt in range(KT):
                        g, j = kt2g[kt]
                        nc.tensor.matmul(
                            ay_ps[:],
                            at_sb[:, kt, :],
                            y_sb[g][:, j, :],
                            start=(kt == 0),
                            stop=(kt == KT - 1),
                        )
                    nc.scalar.activation(
                        out=ay_sb[0:M, :], in_=ay_ps[:],
                        func=mybir.ActivationFunctionType.Copy)

                    # matmul2 + elementwise, per column group
                    for g, (k0, ng) in enumerate(GROUPS):
                        gp = ppool_gp.tile([128, ng, N], F32, tag=f"gp{g}")
                        for j in range(ng):
                            nc.tensor.matmul(
                                gp[:, j, :],
                                w2_sb[:, k0 + j, :],
                                ay_sb[:],
                                start=True,
                                stop=True,
                            )
                        # z = y - mu*(A^T Ay - Atb) = (gp * -mu) + y
                        z = tpool.tile([128, ng, N], F32, tag=f"z{g}")
                        nc.vector.scalar_tensor_tensor(
                            out=z[:],
                            in0=gp[:],
                            scalar=-mu_s,
                            in1=y_sb[g][:],
                            op0=mybir.AluOpType.mult,
                            op1=mybir.AluOpType.add,
                        )
                        # soft threshold: xn = relu(z - thr) - relu(-z - thr)
                        p = tpool.tile([128, ng, N], F32, tag=f"p{g}")
                        q = tpool.tile([128, ng, N], F32, tag=f"q{g}")
                        nc.scalar.activation(
                            out=p[:], in_=z[:],
                            func=mybir.ActivationFunctionType.Relu,
                            bias=negthr[:], scale=1.0,
                        )
                        nc.scalar.activation(
                            out=q[:], in_=z[:],
                            func=mybir.ActivationFunctionType.Relu,
                            bias=negthr[:], scale=-1.0,
                        )
                        xn = x_sb[cur][g]
                        nc.gpsimd.tensor_sub(xn[:], p[:], q[:])
                        # y = xn + coef[i]*(xn - x_prev); coef[0] = 0 makes
                        # the first iteration reduce to y = xn (x_prev = 0)
                        d = tpool.tile([128, ng, N], F32, tag=f"d{g}")
                        nc.gpsimd.tensor_sub(d[:], xn[:], x_sb[prev][g][:])
                        nc.vector.scalar_tensor_tensor(
                            out=y_sb[g][:],
                            in0=d[:],
                            scalar=c_ap,
                            in1=xn[:],
                            op0=mybir.AluOpType.mult,
                            op1=mybir.AluOpType.add,
                        )

            # --- write back final x ------------------------------------
            fin = (iters - 1) % 2
            for g, (off, cnt) in enumerate(GROUPS):
                nc.sync.dma_start(out=xout_d[:, off : off + cnt, :],
                                  in_=x_sb[fin][g][:])

    _legalize_waits(nc)
    return nc


def _prep_inputs(inp, A):
    """Host-side shard/reshape: returns per-core input maps."""
    A = np.asarray(A, np.float32)
    A_pad = np.zeros((M, DP), np.float32)
    A_pad[:, :D] = A
    a_tiles = np.zeros((128, KT, 128), np.float32)
    a_tiles[:M] = A_pad.reshape(M, KT, 128)
    ay_init = np.zeros((128, N), np.float32)
    ay_init[CT0 : CT0 + N] = np.eye(N, dtype=np.float32)
    at_tiles = np.ascontiguousarray(
        A_pad.T.reshape(KT, 128, M).transpose(1, 0, 2))  # [128, KT, M]

    inp = np.asarray(inp, np.float32)
    in_maps = []
    for c in range(NCORES):
        chunk = inp[c * BPC : (c + 1) * BPC]            # (8, 81, 3)
        b_mat = np.ascontiguousarray(chunk.transpose(1, 0, 2).reshape(M, N))
        in_maps.append({"at": at_tiles, "a": a_tiles, "b": b_mat,
                        "ayinit": ay_init})
    return in_maps


def _unshard(results):
    outs = []
    for c in range(NCORES):
        xo = np.asarray(results[c]["xout"])              # [128, KT, N]
        x_dn = xo.transpose(1, 0, 2).reshape(DP, N)[:D]  # (5184, 24)
        outs.append(x_dn.reshape(72, 72, BPC, 3).transpose(2, 0, 1, 3))
    return np.concatenate(outs, 0).astype(np.float32)    # (64, 72, 72, 3)


def _run(inp, A, lam, mu, trace=False):
    mu_s = float(np.asarray(mu).reshape(-1)[0])
    thr = float(np.asarray(lam).reshape(-1)[0]) * mu_s
    key = (mu_s, thr, ITERS)
    if key not in _CACHE:
        _CACHE[key] = _build(mu_s, thr, ITERS)
    nc = _CACHE[key]
    in_maps = _prep_inputs(inp, A)
    res = run_bass_kernel_spmd(nc, in_maps, list(range(NCORES)), trace=trace)
    return _unshard(res.results), res


def kernel(inp, A, lam, mu):
    out, _ = _run(inp, A, lam, mu)
    return out



# revision 41
# speedup vs baseline: 257.9493x; 1.0876x over previous
"""FISTA compressed-sensing kernel for Trainium2 (8 NeuronCores, SPMD).

Problem: for each of 64 patches (x3 channels), run 200 FISTA iterations of
    min_x 0.5||A x - b||^2 + lam||x||_1,   A: (81, 5184)
Sharding: pure data-parallel over the batch — 8 patches x 3 channels = 24
columns per core; A replicated.

Per-core formulation (column matrix Y: (5184, 24)):
    Ay   = A @ Y                      (81, 24)
    G    = A^T @ Ay - Atb             (5184, 24)   [Atb folded into the
                                                    matmul via 24 extra
                                                    contraction rows]
    Z    = Y - mu*G
    Xn   = soft_threshold(Z, lam*mu)
    Y'   = Xn + coef_i * (Xn - X)

Layout: D=5184 padded to 5248 = 41*128; state tiles [128, kt, 24] with
d = kt*128 + p. Ktiles are split into GROUPS so each gradient group fits
one PSUM bank and the per-iteration elementwise tail stays short.

The 200 iterations run as a hardware For_i loop (UNROLL iterations per
trip), with the FISTA momentum coefficient computed on-device via its
scalar recurrence — so the NEFF size is independent of the iteration
count and per-call NEFF-load overhead stays constant.
"""

import os

import numpy as np

import concourse.bass as bass
import concourse.mybir as mybir
import concourse.tile as tile
from concourse.bass_utils import run_bass_kernel_spmd

F32 = mybir.dt.float32

M = 81            # measurements (9x9 camera patch)
D = 5184          # atoms (72x72 upsampled grid)
KT = 41           # 128-row tiles covering D (padded to 5248)
DP = KT * 128     # 5248
NCORES = 8
B = 64
BPC = B // NCORES           # 8 patches per core
N = BPC * 3                 # 24 state columns per core
ITERS = int(os.environ.get("FISTA_ITERS", "200"))
KA = 128                    # augmented contraction dim for matmul2
CT0 = 96                    # partition row where the -Atb^T block starts (32-aligned)
# ktile split into column groups: each gradient group fits one PSUM
# bank (<=512 f32/partition) and groups pipeline matmul2 with elementwise
GROUPS = [(0, 21), (21, 20)]
UNROLL = 4                  # FISTA iterations per hardware-loop trip
SOFTTHRESH = os.environ.get("FISTA_SOFTTHRESH", "relu")  # "relu" | "clip"

_CACHE = {}


def _legalize_waits(nc):
    """This walrus build accepts at most ONE semaphore wait per instruction
    (setupSyncWait: 'Too many sync wait commands'). Tile emits multi-wait
    instructions; split the excess waits onto injected same-engine NoOps
    placed immediately before the instruction (engine queues are FIFO, so
    semantics are identical)."""
    n = 0
    for fn in nc.m.functions:
        for bb in fn.blocks:
            insts = bb.instructions
            out = []
            changed = False
            for ins in insts:
                si = ins.sync_info
                ow = list(si.on_wait) if si is not None else []
                if len(ow) > 1 and ins.engine is not None:
                    for w in ow[:-1]:
                        n += 1
                        out.append(mybir.InstNoOp(
                            name=f"I-waitnop-{n}",
                            engine=ins.engine,
                            ins=[],
                            outs=[],
                            debug=ins.debug,
                            sync_info=mybir.SyncInfo(on_wait=[w], on_update=[]),
                        ))
                    ins.sync_info = mybir.SyncInfo(
                        on_wait=[ow[-1]], on_update=list(si.on_update))
                    changed = True
                out.append(ins)
            if changed:
                bb.instructions = out
    return n


def _build(mu_s, thr, iters):
    """Build the Bass module (same program for all 8 cores)."""
    assert iters % 2 == 0
    nc = bass.Bass()

    # DRAM parameters (per core): A^T tiles, A tiles, measurement matrix b.
    at_d = nc.declare_dram_parameter("at", [128, KT, M], F32, isOutput=False)
    a_d = nc.declare_dram_parameter("a", [128, KT, 128], F32, isOutput=False)
    b_d = nc.declare_dram_parameter("b", [M, N], F32, isOutput=False)
    ay0_d = nc.declare_dram_parameter("ayinit", [128, N], F32, isOutput=False)
    xout_d = nc.declare_dram_parameter("xout", [128, KT, N], F32, isOutput=True)

    with tile.TileContext(nc) as tc:
        with (
            tc.tile_pool(name="weights", bufs=1) as wpool,
            tc.tile_pool(name="state", bufs=1) as spool,
            tc.tile_pool(name="tmp", bufs=2) as tpool,
            tc.tile_pool(name="psum_ay", bufs=2, space="PSUM") as ppool_ay,
            tc.tile_pool(name="psum_gp", bufs=2, space="PSUM") as ppool_gp,
            tc.tile_pool(name="psum_ct", bufs=2, space="PSUM") as ppool_ct,
        ):
            # --- persistent SBUF tensors -------------------------------
            at_sb = wpool.tile([128, KT, M], F32)      # lhsT for matmul1
            w2_sb = wpool.tile([KA, KT, 128], F32)     # lhsT for matmul2
            ay_sb = wpool.tile([KA, N], F32)           # rhs for matmul2
            y_sb = [spool.tile([128, cnt, N], F32, tag=f"y{g}", name=f"y{g}")
                    for g, (off, cnt) in enumerate(GROUPS)]
            # x ping-pong: x_sb[s][g]
            x_sb = [[spool.tile([128, cnt, N], F32, tag=f"x{s}{g}",
                                name=f"x{s}{g}")
                     for g, (off, cnt) in enumerate(GROUPS)]
                    for s in range(2)]

            nc.sync.dma_start(out=at_sb[:], in_=at_d[:])
            nc.sync.dma_start(out=w2_sb[:], in_=a_d[:])

            b_sb = wpool.tile([M, N], F32)
            nc.sync.dma_start(out=b_sb[:], in_=b_d[:])

            negthr = wpool.tile([128, 1], F32)
            nc.vector.memset(negthr[:], -thr)

            # rhs init: zeros with identity block at rows CT0..CT0+N
            nc.sync.dma_start(out=ay_sb[:], in_=ay0_d[:])

            # initial state: x = y = 0
            for g in range(len(GROUPS)):
                nc.vector.memset(y_sb[g][:], 0.0)
                nc.vector.memset(x_sb[1][g][:], 0.0)

            # --- fold -Atb^T into rows CT0..CT0+N of w2 ----------------
            # ct = b^T A  (24, 5248), computed in chunks of 4 ktiles (512),
            # staged in SBUF then written into w2 with ONE instruction so
            # downstream LDWEIGHTS carry few sync waits.
            ct_stage = wpool.tile([N, KT * 128], F32)
            for c4 in range((KT + 3) // 4):
                k0 = c4 * 4
                nk = min(4, KT - k0)
                ct_ps = ppool_ct.tile([N, 4 * 128], F32, tag="ct")
                nc.tensor.matmul(
                    ct_ps[:, : nk * 128],
                    b_sb[:],
                    w2_sb[0:M, k0 : k0 + nk, :].rearrange("k a b -> k (a b)"),
                    start=True,
                    stop=True,
                )
                nc.vector.tensor_scalar_mul(
                    ct_stage[:, k0 * 128 : (k0 + nk) * 128],
                    ct_ps[:, : nk * 128],
                    -1.0,
                )
            nc.vector.tensor_copy(
                w2_sb[CT0 : CT0 + N, :, :].rearrange("k a b -> k (a b)"),
                ct_stage[:],
            )

            # momentum coefficient state for the on-device recurrence:
            # t_{k+1} = (1 + sqrt(1 + 4 t_k^2)) / 2,  c_k = (t_k - 1)/t_{k+1}
            # t ping-pongs by iteration parity; c in its own tile per phase.
            t_sb = [wpool.tile([128, 1], F32, name="t0"),
                    wpool.tile([128, 1], F32, name="t1")]
            c_sb = [wpool.tile([128, 1], F32, name="c0"),
                    wpool.tile([128, 1], F32, name="c1")]
            tmp1 = wpool.tile([128, 1], F32, name="tmp1")
            nc.vector.memset(t_sb[0][:], 1.0)

            # ktile -> group index lookup
            kt2g = {}
            for g, (off, cnt) in enumerate(GROUPS):
                for j in range(cnt):
                    kt2g[off + j] = (g, j)

            # --- FISTA iterations: hardware loop, UNROLL per trip ------
            unroll = UNROLL if iters % UNROLL == 0 else 2
            assert iters % unroll == 0
            with tc.For_i(0, iters, unroll,
                          hint_engines=(mybir.EngineType.PE,)) as it:
                for phase in range(unroll):
                    cur, prev = phase % 2, 1 - phase % 2
                    tp, tn = t_sb[cur], t_sb[prev]
                    # coef chain (tiny [128,1] ops, off the critical path):
                    # tmp1 = 4*t^2 + 1 ; tn = (sqrt(tmp1)+1)/2 ;
                    # tmp1 = 1/tn ; c = (t-1)*tmp1
                    nc.vector.tensor_scalar(
                        tmp1[:], tp[:], tp[:], None, mybir.AluOpType.mult)
                    nc.scalar.activation(
                        out=tn[:], in_=tmp1[:],
                        func=mybir.ActivationFunctionType.Sqrt,
                        bias=1.0, scale=4.0)
                    nc.vector.tensor_scalar(
                        tn[:], tn[:], 1.0, 0.5,
                        mybir.AluOpType.add, mybir.AluOpType.mult)
                    nc.vector.reciprocal(tmp1[:], tn[:])
                    nc.vector.scalar_tensor_tensor(
                        out=c_sb[cur][:], in0=tp[:], scalar=1.0,
                        in1=tmp1[:],
                        op0=mybir.AluOpType.subtract,
                        op1=mybir.AluOpType.mult)
                    c_ap = c_sb[cur][:]

                    # matmul1: Ay = A @ Y  -> psum (81, 24)
                    ay_ps = ppool_ay.tile([M, N], F32, tag="ay")
                    for kt in range(KT):
                        g, j = kt2g[kt]
                        nc.tensor.matmul(
                            ay_ps[:],
                            at_sb[:, kt, :],
                            y_sb[g][:, j, :],
                            start=(kt == 0),
                            stop=(kt == KT - 1),
                        )
                    nc.scalar.activation(
                        out=ay_sb[0:M, :], in_=ay_ps[:],
                        func=mybir.ActivationFunctionType.Copy)

                    # matmul2 + elementwise, per column group
                    for g, (k0, ng) in enumerate(GROUPS):
                        gp = ppool_gp.tile([128, ng, N], F32, tag=f"gp{g}")
                        for j in range(ng):
                            nc.tensor.matmul(
                                gp[:, j, :],
                                w2_sb[:, k0 + j, :],
                                ay_sb[:],
                                start=True,
                                stop=True,
                            )
                        # z = y - mu*(A^T Ay - Atb) = (gp * -mu) + y
                        z = tpool.tile([128, ng, N], F32, tag=f"z{g}")
                        nc.vector.scalar_tensor_tensor(
                            out=z[:],
                            in0=gp[:],
                            scalar=-mu_s,
                            in1=y_sb[g][:],
                            op0=mybir.AluOpType.mult,
                            op1=mybir.AluOpType.add,
                        )
                        xn = x_sb[cur][g]
                        if SOFTTHRESH == "clip":
                            # soft threshold: xn = z - clip(z, -thr, +thr)
                            cl = tpool.tile([128, ng, N], F32, tag=f"cl{g}")
                            nc.vector.tensor_scalar(
                                cl[:], z[:], -thr, thr,
                                mybir.AluOpType.max, mybir.AluOpType.min)
                            nc.vector.tensor_sub(xn[:], z[:], cl[:])
                        else:
                            # xn = relu(z - thr) - relu(-z - thr)
                            p = tpool.tile([128, ng, N], F32, tag=f"p{g}")
                            q = tpool.tile([128, ng, N], F32, tag=f"q{g}")
                            nc.scalar.activation(
                                out=p[:], in_=z[:],
                                func=mybir.ActivationFunctionType.Relu,
                                bias=negthr[:], scale=1.0)
                            nc.scalar.activation(
                                out=q[:], in_=z[:],
                                func=mybir.ActivationFunctionType.Relu,
                                bias=negthr[:], scale=-1.0)
                            nc.gpsimd.tensor_sub(xn[:], p[:], q[:])
                        # y = xn + coef[i]*(xn - x_prev); coef[0] = 0 makes
                        # the first iteration reduce to y = xn (x_prev = 0)
                        d = tpool.tile([128, ng, N], F32, tag=f"d{g}")
                        if SOFTTHRESH == "clip":
                            nc.vector.tensor_sub(d[:], xn[:], x_sb[prev][g][:])
                        else:
                            nc.gpsimd.tensor_sub(d[:], xn[:], x_sb[prev][g][:])
                        nc.vector.scalar_tensor_tensor(
                            out=y_sb[g][:],
                            in0=d[:],
                            scalar=c_ap,
                            in1=xn[:],
                            op0=mybir.AluOpType.mult,
                            op1=mybir.AluOpType.add,
                        )

            # --- write back final x ------------------------------------
            fin = (iters - 1) % 2
            for g, (off, cnt) in enumerate(GROUPS):
                nc.sync.dma_start(out=xout_d[:, off : off + cnt, :],
                                  in_=x_sb[fin][g][:])

    _legalize_waits(nc)
    return nc


def _prep_inputs(inp, A):
    """Host-side shard/reshape: returns per-core input maps."""
    A = np.asarray(A, np.float32)
    A_pad = np.zeros((M, DP), np.float32)
    A_pad[:, :D] = A
    a_tiles = np.zeros((128, KT, 128), np.float32)
    a_tiles[:M] = A_pad.reshape(M, KT, 128)
    ay_init = np.zeros((128, N), np.float32)
    ay_init[CT0 : CT0 + N] = np.eye(N, dtype=np.float32)
    at_tiles = np.ascontiguousarray(
        A_pad.T.reshape(KT, 128, M).transpose(1, 0, 2))  # [128, KT, M]

    inp = np.asarray(inp, np.float32)
    in_maps = []
    for c in range(NCORES):
        chunk = inp[c * BPC : (c + 1) * BPC]            # (8, 81, 3)
        b_mat = np.ascontiguousarray(chunk.transpose(1, 0, 2).reshape(M, N))
        in_maps.append({"at": at_tiles, "a": a_tiles, "b": b_mat,
                        "ayinit": ay_init})
    return in_maps


def _unshard(results):
    outs = []
    for c in range(NCORES):
        xo = np.asarray(results[c]["xout"])              # [128, KT, N]
        x_dn = xo.transpose(1, 0, 2).reshape(DP, N)[:D]  # (5184, 24)
        outs.append(x_dn.reshape(72, 72, BPC, 3).transpose(2, 0, 1, 3))
    return np.concatenate(outs, 0).astype(np.float32)    # (64, 72, 72, 3)


def _run(inp, A, lam, mu, trace=False):
    mu_s = float(np.asarray(mu).reshape(-1)[0])
    thr = float(np.asarray(lam).reshape(-1)[0]) * mu_s
    key = (mu_s, thr, ITERS)
    if key not in _CACHE:
        _CACHE[key] = _build(mu_s, thr, ITERS)
    nc = _CACHE[key]
    in_maps = _prep_inputs(inp, A)
    res = run_bass_kernel_spmd(nc, in_maps, list(range(NCORES)), trace=trace)
    return _unshard(res.results), res


def kernel(inp, A, lam, mu):
    out, _ = _run(inp, A, lam, mu)
    return out



# revision 47
# speedup vs baseline: 258.0308x; 1.0003x over previous
"""FISTA compressed-sensing kernel for Trainium2 (8 NeuronCores, SPMD).

Problem: for each of 64 patches (x3 channels), run 200 FISTA iterations of
    min_x 0.5||A x - b||^2 + lam||x||_1,   A: (81, 5184)
Sharding: pure data-parallel over the batch — 8 patches x 3 channels = 24
columns per core; A replicated.

Per-core formulation (column matrix Y: (5184, 24)):
    Ay   = A @ Y                      (81, 24)
    G    = A^T @ Ay - Atb             (5184, 24)   [Atb folded into the
                                                    matmul via 24 extra
                                                    contraction rows]
    Z    = Y - mu*G
    Xn   = soft_threshold(Z, lam*mu)
    Y'   = Xn + coef_i * (Xn - X)

Layout: D=5184 padded to 5248 = 41*128; state tiles [128, kt, 24] with
d = kt*128 + p. Ktiles are split into GROUPS so each gradient group fits
one PSUM bank and the per-iteration elementwise tail stays short.

The 200 iterations run as a hardware For_i loop (UNROLL iterations per
trip), with the FISTA momentum coefficient computed on-device via its
scalar recurrence — so the NEFF size is independent of the iteration
count and per-call NEFF-load overhead stays constant.
"""

import os

import numpy as np

import concourse.bass as bass
import concourse.mybir as mybir
import concourse.tile as tile
from concourse.bass_utils import run_bass_kernel_spmd

F32 = mybir.dt.float32

M = 81            # measurements (9x9 camera patch)
D = 5184          # atoms (72x72 upsampled grid)
KT = 41           # 128-row tiles covering D (padded to 5248)
DP = KT * 128     # 5248
NCORES = 8
B = 64
BPC = B // NCORES           # 8 patches per core
N = BPC * 3                 # 24 state columns per core
ITERS = int(os.environ.get("FISTA_ITERS", "200"))
KA = 128                    # augmented contraction dim for matmul2
CT0 = 96                    # partition row where the -Atb^T block starts (32-aligned)
# ktile split into column groups: each gradient group fits one PSUM
# bank (<=512 f32/partition) and groups pipeline matmul2 with elementwise
GROUPS = [(0, 21), (21, 20)]
UNROLL = 4                  # FISTA iterations per hardware-loop trip
SOFTTHRESH = os.environ.get("FISTA_SOFTTHRESH", "relu")  # "relu" | "clip"

_CACHE = {}


def _legalize_waits(nc):
    """This walrus build accepts at most ONE semaphore wait per instruction
    (setupSyncWait: 'Too many sync wait commands'). Tile emits multi-wait
    instructions; split the excess waits onto injected same-engine NoOps
    placed immediately before the instruction (engine queues are FIFO, so
    semantics are identical)."""
    n = 0
    for fn in nc.m.functions:
        for bb in fn.blocks:
            insts = bb.instructions
            out = []
            changed = False
            for ins in insts:
                si = ins.sync_info
                ow = list(si.on_wait) if si is not None else []
                if len(ow) > 1 and ins.engine is not None:
                    for w in ow[:-1]:
                        n += 1
                        out.append(mybir.InstNoOp(
                            name=f"I-waitnop-{n}",
                            engine=ins.engine,
                            ins=[],
                            outs=[],
                            debug=ins.debug,
                            sync_info=mybir.SyncInfo(on_wait=[w], on_update=[]),
                        ))
                    ins.sync_info = mybir.SyncInfo(
                        on_wait=[ow[-1]], on_update=list(si.on_update))
                    changed = True
                out.append(ins)
            if changed:
                bb.instructions = out
    return n


def _strip_pe_updates(nc):  # UNUSED — kept as documentation of a negative result
    """Drop PE semaphore increments nobody waits on.

    MEASURED OUTCOME (2026-08-08): stripping 316/328 per-matmul PE sem
    increments (with wait renumbering) builds and compiles but HANGS at
    execution — the For_i reset/drain machinery evidently depends on the
    full per-instruction tick counts. Do not re-enable without solving
    that; the potential upside was only ~2us/iter (26ns/EVT_SEM write).

    Tile ticks each engine's semaphore on EVERY instruction; the loop body
    has ~328 PE matmuls but only ~17 PE-tick values are ever waited on
    (per-trip numbering — the loop reset zeroes the sems each trip).
    Each EVT_SEM write serializes on the engine, so the unwaited ticks are
    pure overhead. Keep updates only at waited ticks and renumber every
    wait on that semaphore (new_v = v - dropped_before_or_at(v)).
    Only the PE engine semaphore inside the For_i body is touched; waits
    referencing it in the reset/exit blocks use the same per-trip
    numbering and are renumbered identically.
    """
    for fn in nc.m.functions:
        body = [bb for bb in fn.blocks if bb.name.endswith("_body")]
        if len(body) != 1:
            continue
        body = body[0]
        # the PE engine semaphore: the one every PE instruction increments
        pe_sem = None
        for ins in body.instructions:
            if ins.engine == mybir.EngineType.PE and ins.sync_info:
                for u in ins.sync_info.on_update:
                    if u.sync_type == "semaphore" and u.update_mode == "sem-inc":
                        pe_sem = u.id
                        break
            if pe_sem is not None:
                break
        if pe_sem is None:
            continue
        # Per-trip tick numbering applies inside the loop machinery blocks
        # only (the reset zeroes the sems at loop entry and per back-edge);
        # the prologue uses its own pre-reset numbering and the post-loop
        # blocks are uncertain — bail if they reference the PE semaphore.
        loop_tag = body.name.rsplit("_body", 1)[0]
        in_loop = lambda bb: bb.name.startswith(loop_tag)
        waited = set()
        bail = False
        for bb in fn.blocks:
            for ins in bb.instructions:
                if ins.sync_info:
                    for w in ins.sync_info.on_wait:
                        if (w.sync_type == "semaphore" and w.id == pe_sem
                                and w.wait_mode == "sem-ge-imm"):
                            if in_loop(bb):
                                waited.add(w.wait_value)
                            elif "_build_end" in bb.name or "after_loop" in bb.name:
                                bail = True
        if bail:
            continue
        # walk body PE updates in order; drop increments at unwaited ticks
        tick = 0
        dropped_upto = {}  # original tick -> #dropped with tick' <= tick
        ndrop = 0
        for ins in body.instructions:
            si = ins.sync_info
            if si is None:
                continue
            keep = []
            for u in si.on_update:
                if (u.sync_type == "semaphore" and u.id == pe_sem
                        and u.update_mode == "sem-inc"):
                    tick += u.update_value
                    if tick in waited:
                        keep.append(u)
                    else:
                        ndrop += u.update_value
                    dropped_upto[tick] = ndrop
                else:
                    keep.append(u)
            if len(keep) != len(si.on_update):
                ins.sync_info = mybir.SyncInfo(
                    on_wait=list(si.on_wait), on_update=keep)
        if ndrop == 0:
            continue
        # renumber waits on the PE semaphore in the loop machinery blocks
        for bb in fn.blocks:
            if not in_loop(bb):
                continue
            for ins in bb.instructions:
                si = ins.sync_info
                if si is None:
                    continue
                changed = False
                new_waits = []
                for w in si.on_wait:
                    if (w.sync_type == "semaphore" and w.id == pe_sem
                            and w.wait_mode == "sem-ge-imm"
                            and w.wait_value in dropped_upto):
                        new_waits.append(mybir.SyncWait(
                            sync_type=w.sync_type, id=w.id,
                            ant_name=w.ant_name, wait_mode=w.wait_mode,
                            wait_value=w.wait_value - dropped_upto[w.wait_value],
                            wait_reg=w.wait_reg))
                        changed = True
                    else:
                        new_waits.append(w)
                if changed:
                    ins.sync_info = mybir.SyncInfo(
                        on_wait=new_waits, on_update=list(si.on_update))
    return


def _build(mu_s, thr, iters):
    """Build the Bass module (same program for all 8 cores)."""
    assert iters % 2 == 0
    nc = bass.Bass()

    # DRAM parameters (per core): A^T tiles, A tiles, measurement matrix b.
    at_d = nc.declare_dram_parameter("at", [128, KT, M], F32, isOutput=False)
    a_d = nc.declare_dram_parameter("a", [128, KT, 128], F32, isOutput=False)
    b_d = nc.declare_dram_parameter("b", [M, N], F32, isOutput=False)
    ay0_d = nc.declare_dram_parameter("ayinit", [128, N], F32, isOutput=False)
    xout_d = nc.declare_dram_parameter("xout", [128, KT, N], F32, isOutput=True)

    with tile.TileContext(nc) as tc:
        with (
            tc.tile_pool(name="weights", bufs=1) as wpool,
            tc.tile_pool(name="state", bufs=1) as spool,
            tc.tile_pool(name="tmp", bufs=2) as tpool,
            tc.tile_pool(name="psum_ay", bufs=2, space="PSUM") as ppool_ay,
            tc.tile_pool(name="psum_gp", bufs=2, space="PSUM") as ppool_gp,
            tc.tile_pool(name="psum_ct", bufs=2, space="PSUM") as ppool_ct,
        ):
            # --- persistent SBUF tensors -------------------------------
            at_sb = wpool.tile([128, KT, M], F32)      # lhsT for matmul1
            w2_sb = wpool.tile([KA, KT, 128], F32)     # lhsT for matmul2
            ay_sb = wpool.tile([KA, N], F32)           # rhs for matmul2
            y_sb = [spool.tile([128, cnt, N], F32, tag=f"y{g}", name=f"y{g}")
                    for g, (off, cnt) in enumerate(GROUPS)]
            # x ping-pong: x_sb[s][g]
            x_sb = [[spool.tile([128, cnt, N], F32, tag=f"x{s}{g}",
                                name=f"x{s}{g}")
                     for g, (off, cnt) in enumerate(GROUPS)]
                    for s in range(2)]

            nc.sync.dma_start(out=at_sb[:], in_=at_d[:])
            nc.sync.dma_start(out=w2_sb[:], in_=a_d[:])

            b_sb = wpool.tile([M, N], F32)
            nc.sync.dma_start(out=b_sb[:], in_=b_d[:])

            negthr = wpool.tile([128, 1], F32)
            nc.vector.memset(negthr[:], -thr)

            # rhs init: zeros with identity block at rows CT0..CT0+N
            nc.sync.dma_start(out=ay_sb[:], in_=ay0_d[:])

            # initial state: x = y = 0
            for g in range(len(GROUPS)):
                nc.vector.memset(y_sb[g][:], 0.0)
                nc.vector.memset(x_sb[1][g][:], 0.0)

            # --- fold -Atb^T into rows CT0..CT0+N of w2 ----------------
            # ct = b^T A  (24, 5248), computed in chunks of 4 ktiles (512),
            # staged in SBUF then written into w2 with ONE instruction so
            # downstream LDWEIGHTS carry few sync waits.
            ct_stage = wpool.tile([N, KT * 128], F32)
            for c4 in range((KT + 3) // 4):
                k0 = c4 * 4
                nk = min(4, KT - k0)
                ct_ps = ppool_ct.tile([N, 4 * 128], F32, tag="ct")
                nc.tensor.matmul(
                    ct_ps[:, : nk * 128],
                    b_sb[:],
                    w2_sb[0:M, k0 : k0 + nk, :].rearrange("k a b -> k (a b)"),
                    start=True,
                    stop=True,
                )
                nc.vector.tensor_scalar_mul(
                    ct_stage[:, k0 * 128 : (k0 + nk) * 128],
                    ct_ps[:, : nk * 128],
                    -1.0,
                )
            nc.vector.tensor_copy(
                w2_sb[CT0 : CT0 + N, :, :].rearrange("k a b -> k (a b)"),
                ct_stage[:],
            )

            # momentum coefficient state for the on-device recurrence:
            # t_{k+1} = (1 + sqrt(1 + 4 t_k^2)) / 2,  c_k = (t_k - 1)/t_{k+1}
            # t ping-pongs by iteration parity; c in its own tile per phase.
            t_sb = [wpool.tile([128, 1], F32, name="t0"),
                    wpool.tile([128, 1], F32, name="t1")]
            c_sb = [wpool.tile([128, 1], F32, name="c0"),
                    wpool.tile([128, 1], F32, name="c1")]
            tmp1 = wpool.tile([128, 1], F32, name="tmp1")
            nc.vector.memset(t_sb[0][:], 1.0)

            # ktile -> group index lookup
            kt2g = {}
            for g, (off, cnt) in enumerate(GROUPS):
                for j in range(cnt):
                    kt2g[off + j] = (g, j)

            # --- FISTA iterations: hardware loop, UNROLL per trip ------
            unroll = UNROLL if iters % UNROLL == 0 else 2
            assert iters % unroll == 0
            with tc.For_i(0, iters, unroll,
                          hint_engines=(mybir.EngineType.PE,)) as it:
                for phase in range(unroll):
                    cur, prev = phase % 2, 1 - phase % 2
                    tp, tn = t_sb[cur], t_sb[prev]
                    # coef chain (tiny [128,1] ops, off the critical path):
                    # tmp1 = 4*t^2 + 1 ; tn = (sqrt(tmp1)+1)/2 ;
                    # tmp1 = 1/tn ; c = (t-1)*tmp1
                    nc.vector.tensor_scalar(
                        tmp1[:], tp[:], tp[:], None, mybir.AluOpType.mult)
                    nc.scalar.activation(
                        out=tn[:], in_=tmp1[:],
                        func=mybir.ActivationFunctionType.Sqrt,
                        bias=1.0, scale=4.0)
                    nc.vector.tensor_scalar(
                        tn[:], tn[:], 1.0, 0.5,
                        mybir.AluOpType.add, mybir.AluOpType.mult)
                    nc.vector.reciprocal(tmp1[:], tn[:])
                    nc.vector.scalar_tensor_tensor(
                        out=c_sb[cur][:], in0=tp[:], scalar=1.0,
                        in1=tmp1[:],
                        op0=mybir.AluOpType.subtract,
                        op1=mybir.AluOpType.mult)
                    c_ap = c_sb[cur][:]

                    # matmul1: Ay = A @ Y  -> psum (81, 24)
                    ay_ps = ppool_ay.tile([M, N], F32, tag="ay")
                    for kt in range(KT):
                        g, j = kt2g[kt]
                        nc.tensor.matmul(
                            ay_ps[:],
                            at_sb[:, kt, :],
                            y_sb[g][:, j, :],
                            start=(kt == 0),
                            stop=(kt == KT - 1),
                        )
                    nc.scalar.activation(
                        out=ay_sb[0:M, :], in_=ay_ps[:],
                        func=mybir.ActivationFunctionType.Copy)

                    # matmul2 + elementwise, per column group
                    for g, (k0, ng) in enumerate(GROUPS):
                        gp = ppool_gp.tile([128, ng, N], F32, tag=f"gp{g}")
                        for j in range(ng):
                            nc.tensor.matmul(
                                gp[:, j, :],
                                w2_sb[:, k0 + j, :],
                                ay_sb[:],
                                start=True,
                                stop=True,
                            )
                        # z = y - mu*(A^T Ay - Atb) = (gp * -mu) + y
                        z = tpool.tile([128, ng, N], F32, tag=f"z{g}")
                        nc.vector.scalar_tensor_tensor(
                            out=z[:],
                            in0=gp[:],
                            scalar=-mu_s,
                            in1=y_sb[g][:],
                            op0=mybir.AluOpType.mult,
                            op1=mybir.AluOpType.add,
                        )
                        xn = x_sb[cur][g]
                        if SOFTTHRESH == "clip":
                            # soft threshold: xn = z - clip(z, -thr, +thr)
                            cl = tpool.tile([128, ng, N], F32, tag=f"cl{g}")
                            nc.vector.tensor_scalar(
                                cl[:], z[:], -thr, thr,
                                mybir.AluOpType.max, mybir.AluOpType.min)
                            nc.vector.tensor_sub(xn[:], z[:], cl[:])
                        else:
                            # xn = relu(z - thr) - relu(-z - thr)
                            p = tpool.tile([128, ng, N], F32, tag=f"p{g}")
                            q = tpool.tile([128, ng, N], F32, tag=f"q{g}")
                            nc.scalar.activation(
                                out=p[:], in_=z[:],
                                func=mybir.ActivationFunctionType.Relu,
                                bias=negthr[:], scale=1.0)
                            nc.scalar.activation(
                                out=q[:], in_=z[:],
                                func=mybir.ActivationFunctionType.Relu,
                                bias=negthr[:], scale=-1.0)
                            nc.gpsimd.tensor_sub(xn[:], p[:], q[:])
                        # y = xn + coef[i]*(xn - x_prev); coef[0] = 0 makes
                        # the first iteration reduce to y = xn (x_prev = 0)
                        d = tpool.tile([128, ng, N], F32, tag=f"d{g}")
                        if SOFTTHRESH == "clip":
                            nc.vector.tensor_sub(d[:], xn[:], x_sb[prev][g][:])
                        else:
                            nc.gpsimd.tensor_sub(d[:], xn[:], x_sb[prev][g][:])
                        nc.vector.scalar_tensor_tensor(
                            out=y_sb[g][:],
                            in0=d[:],
                            scalar=c_ap,
                            in1=xn[:],
                            op0=mybir.AluOpType.mult,
                            op1=mybir.AluOpType.add,
                        )

            # --- write back final x ------------------------------------
            fin = (iters - 1) % 2
            for g, (off, cnt) in enumerate(GROUPS):
                nc.sync.dma_start(out=xout_d[:, off : off + cnt, :],
                                  in_=x_sb[fin][g][:])

    _legalize_waits(nc)
    return nc


def _prep_inputs(inp, A):
    """Host-side shard/reshape: returns per-core input maps."""
    A = np.asarray(A, np.float32)
    A_pad = np.zeros((M, DP), np.float32)
    A_pad[:, :D] = A
    a_tiles = np.zeros((128, KT, 128), np.float32)
    a_tiles[:M] = A_pad.reshape(M, KT, 128)
    ay_init = np.zeros((128, N), np.float32)
    ay_init[CT0 : CT0 + N] = np.eye(N, dtype=np.float32)
    at_tiles = np.ascontiguousarray(
        A_pad.T.reshape(KT, 128, M).transpose(1, 0, 2))  # [128, KT, M]

    inp = np.asarray(inp, np.float32)
    in_maps = []
    for c in range(NCORES):
        chunk = inp[c * BPC : (c + 1) * BPC]            # (8, 81, 3)
        b_mat = np.ascontiguousarray(chunk.transpose(1, 0, 2).reshape(M, N))
        in_maps.append({"at": at_tiles, "a": a_tiles, "b": b_mat,
                        "ayinit": ay_init})
    return in_maps


def _unshard(results):
    outs = []
    for c in range(NCORES):
        xo = np.asarray(results[c]["xout"])              # [128, KT, N]
        x_dn = xo.transpose(1, 0, 2).reshape(DP, N)[:D]  # (5184, 24)
        outs.append(x_dn.reshape(72, 72, BPC, 3).transpose(2, 0, 1, 3))
    return np.concatenate(outs, 0).astype(np.float32)    # (64, 72, 72, 3)


def _run(inp, A, lam, mu, trace=False):
    mu_s = float(np.asarray(mu).reshape(-1)[0])
    thr = float(np.asarray(lam).reshape(-1)[0]) * mu_s
    key = (mu_s, thr, ITERS)
    if key not in _CACHE:
        _CACHE[key] = _build(mu_s, thr, ITERS)
    nc = _CACHE[key]
    in_maps = _prep_inputs(inp, A)
    res = run_bass_kernel_spmd(nc, in_maps, list(range(NCORES)), trace=trace)
    return _unshard(res.results), res


def kernel(inp, A, lam, mu):
    out, _ = _run(inp, A, lam, mu)
    return out

